# revision 1
# baseline (speedup 1.0000x reference)
"""Trainium2 Bass kernel for nn_ActionModule (sparse attention, 8 cores).

Sharding: data-parallel over spatial axis S (1560 = 8 x 195 per core).
Each core processes (T=16 frames x 195 spatial) = 3120 tokens through both
branches; small weights replicated; attention is over T=16 frames only.

Per-core tiling: 25 row-tiles of 128 rows; tile = 16 frames x 8 spatial
positions (row p = t*8 + u). The last tile overlaps the previous one so all
tiles are full 128 rows (overlap rows compute identical values).

Phases (per core):
  P0: keyboard cond MLP -> windows -> k2/v2 (tiny, frame-major)
  P1: gm = [x | mouse windows] -> mm MLP -> LayerNorm -> h (SBUF, bf16)
  P2: h -> qkv -> RMS+rope -> local attention (T=16) -> proj -> x_upd (DRAM)
  P3: x_upd -> key_q -> RMS+rope -> cross attention vs k2/v2 -> proj -> out

Matmuls run as float32r (full-rate fp32, ~1e-4 rel err) for the trunk and
bf16 for attention internals / output projections. LN gain/bias and RMS
gains are folded into weights / rope tables on the host.
"""
import sys
sys.path.insert(0, '/opt/trn_rl_repo')

import numpy as np
import ml_dtypes

import bass_rust
import concourse.bass as bass
import concourse.bacc as bacc

# Prefer activation-table set 6 (Ln+Exp+Square together): hide Exp/Ln from
# other sets so the table-load insertion pass never thrashes between the
# exp-only and ln-only tables. The ids passed to walrus stay valid (set 6
# really contains all three); we only constrain the chooser.
_orig_get_tables = bacc.get_activation_tables

def _patched_tables(arch):
    tabs = dict(_orig_get_tables(arch))
    out = {}
    for i, (name, funcs) in enumerate(tabs.items()):
        if name != "natural_log_exp_and_others":
            funcs = {f for f in funcs
                     if f not in (mybir.ActivationFunctionType.Exp,
                                  mybir.ActivationFunctionType.Ln)}
        out[name] = set(funcs)
    return out

bacc.get_activation_tables = _patched_tables
import concourse.bacc as _b2
_b2.get_activation_tables = _patched_tables
import concourse.tile as tile
import concourse.mybir as mybir
from concourse.bass_utils import run_bass_kernel_spmd

f32 = mybir.dt.float32
f32r = mybir.dt.float32r
bf16 = mybir.dt.bfloat16
Alu = mybir.AluOpType
Act = mybir.ActivationFunctionType
AxX = mybir.AxisListType.X

# dims (hardcoded per spec)
T, TH, TW = 16, 30, 52
S = TH * TW            # 1560
SPC = S // 8           # 195 per core
C = 1536               # IMG
CM = 1024
CK = 1024
HID = 128
HD = 64
H = 16                 # heads
PAD_T = 12             # RATIO*WIN
NF = 61
LOCAL = 6
THETA = 256.0
SCALE = 1.0 / 8.0      # 1/sqrt(64)

N_TILES = 25
SCH = 8                # spatial positions per tile
ROWS = T * SCH         # 128
S_STARTS = [min(j * SCH, SPC - SCH) for j in range(N_TILES)]


def _rope_tables():
    t = np.arange(T, dtype=np.float32)
    freqs = 1.0 / (THETA ** (np.arange(0, 8, 2, dtype=np.float32) / 8.0))
    ang = t[:, None] * freqs[None, :]
    cos = np.concatenate([np.cos(ang), np.ones((T, 28), np.float32)], axis=1)
    sin = np.concatenate([np.sin(ang), np.zeros((T, 28), np.float32)], axis=1)
    c_exp = np.repeat(cos, 2, axis=1)   # (16, 64): cos[t, d//2]
    s_exp = np.repeat(sin, 2, axis=1)
    return c_exp, s_exp


def _cg_sg(gain, frame_of_row):
    """CG (R,64): gain[d]*cos[t(p),d] (shared across heads).
    SG (R,8): coeffs for swapped rope pairs:
      SG[p, 2i]   = -gain[2i+1] * sin[t, 2i]
      SG[p, 2i+1] = +gain[2i]   * sin[t, 2i]
    """
    c_exp, s_exp = _rope_tables()
    cg = (gain[None, :] * c_exp[frame_of_row]).astype(np.float32)   # (R, 64)
    sg = np.zeros((len(frame_of_row), 8), np.float32)
    se = s_exp[frame_of_row]
    for i in range(4):
        sg[:, 2 * i] = -gain[2 * i + 1] * se[:, 2 * i]
        sg[:, 2 * i + 1] = gain[2 * i] * se[:, 2 * i + 1]
    return np.ascontiguousarray(cg), np.ascontiguousarray(sg)


def _build_consts(inp):
    c = {}
    frame_of_row = (np.arange(ROWS) // SCH).astype(np.int64)

    c["cgq"], _ = _cg_sg(np.asarray(inp["mq_norm_w"], np.float32), frame_of_row)
    c["cgk"], _ = _cg_sg(np.asarray(inp["mk_norm_w"], np.float32), frame_of_row)
    c["cgq2"], _ = _cg_sg(np.asarray(inp["kq_norm_w"], np.float32), frame_of_row)
    fr16 = np.arange(T, dtype=np.int64)
    c["cg16"], _ = _cg_sg(np.asarray(inp["kk_norm_w"], np.float32), fr16)
    # tan tables: rope correction on the CG-multiplied values; gains cancel:
    #   tmp[2i]   = -tan(ang_i) * qcg[2i+1],  tmp[2i+1] = tan(ang_i) * qcg[2i]
    c_exp, s_exp = _rope_tables()
    for nm, fr in (("tanx", frame_of_row), ("tanx16", fr16)):
        tg = np.zeros((len(fr), 8), np.float32)
        for i in range(4):
            tn = s_exp[fr, 2 * i] / c_exp[fr, 2 * i]
            tg[:, 2 * i] = -tn
            tg[:, 2 * i + 1] = tn
        c[nm] = np.tile(tg, (1, 8)).astype(ml_dtypes.bfloat16)  # (R, 64)

    t_p = frame_of_row
    u_p = np.arange(ROWS) % SCH
    same_s = u_p[:, None] == u_p[None, :]
    near_t = np.abs(t_p[:, None] - t_p[None, :]) <= LOCAL
    c["madd"] = np.where(same_s & near_t, 0.0, -1e9).astype(np.float32)
    near2 = np.abs(t_p[:, None] - fr16[None, :]) <= LOCAL
    c["madd2"] = np.where(near2, 0.0, -1e9).astype(np.float32)

    mc = np.asarray(inp["mouse_condition"], np.float32)[0]  # (61, 2)
    gm24t = np.zeros((PAD_T * 2, ROWS), np.float32)
    for w in range(PAD_T):
        src = np.maximum(4 * frame_of_row + w - PAD_T, 0)
        gm24t[2 * w] = mc[src, 0]
        gm24t[2 * w + 1] = mc[src, 1]
    c["gm24t"] = gm24t

    # LN fold into qkv weights: W' = diag(g) @ W ; b' = b @ W
    g = np.asarray(inp["mm_ln_g"], np.float32)
    b = np.asarray(inp["mm_ln_b"], np.float32)
    wqkv = np.asarray(inp["tqkv_w"], np.float32)
    c["wqkv"] = (g[:, None] * wqkv).astype(ml_dtypes.bfloat16)
    c["bqkv"] = (b @ wqkv).reshape(1, 3 * CM).astype(ml_dtypes.bfloat16)

    c["w1"] = np.asarray(inp["mm_w1"], np.float32)
    c["w2"] = np.asarray(inp["mm_w2"], np.float32).astype(ml_dtypes.bfloat16)
    c["b1r"] = np.asarray(inp["mm_b1"], np.float32).reshape(1, CM)
    c["b2r"] = np.asarray(inp["mm_b2"], np.float32).reshape(1, CM).astype(ml_dtypes.bfloat16)
    c["wpm"] = np.asarray(inp["proj_mouse_w"], np.float32).astype(ml_dtypes.bfloat16)
    c["wkq"] = np.asarray(inp["key_q_w"], np.float32).astype(ml_dtypes.bfloat16)
    c["wkkv"] = np.asarray(inp["key_kv_w"], np.float32)
    c["wpk"] = np.asarray(inp["proj_keyboard_w"], np.float32).astype(ml_dtypes.bfloat16)
    c["kbw1"] = np.asarray(inp["kb_w1"], np.float32)
    c["kbw2"] = np.asarray(inp["kb_w2"], np.float32)
    c["kb1c"] = np.asarray(inp["kb_b1"], np.float32).reshape(HID, 1)
    c["kb2c"] = np.asarray(inp["kb_b2"], np.float32).reshape(HID, 1)
    c["condt"] = np.ascontiguousarray(
        np.asarray(inp["keyboard_condition"], np.float32)[0].T)

    c["ones"] = np.ones((1, 128), np.float32)
    c["onesb"] = np.ones((1, 128), np.float32).astype(ml_dtypes.bfloat16)
    c["ident"] = np.eye(128, dtype=np.float32)
    c["identb"] = np.eye(128, dtype=np.float32).astype(ml_dtypes.bfloat16)
    return c


CONST_SPECS = [
    ("cgq", (ROWS, HD), f32),
    ("cgk", (ROWS, HD), f32),
    ("cgq2", (ROWS, HD), f32),
    ("cg16", (T, HD), f32),
    ("tanx", (ROWS, HD), bf16), ("tanx16", (T, HD), bf16),
    ("madd", (ROWS, ROWS), f32), ("madd2", (ROWS, T), f32),
    ("gm24t", (24, ROWS), f32),
    ("wqkv", (CM, 3 * CM), bf16), ("bqkv", (1, 3 * CM), bf16),
    ("w1", (1560, CM), f32), ("w2", (CM, CM), bf16),
    ("b1r", (1, CM), f32), ("b2r", (1, CM), bf16),
    ("wpm", (CM, C), bf16),
    ("wkq", (C, CK), bf16), ("wkkv", (C, 2 * CK), f32),
    ("wpk", (CK, C), bf16),
    ("kbw1", (6, HID), f32), ("kbw2", (HID, HID), f32),
    ("kb1c", (HID, 1), f32), ("kb2c", (HID, 1), f32),
    ("condt", (6, NF), f32),
    ("ones", (1, 128), f32), ("onesb", (1, 128), bf16),
    ("ident", (128, 128), f32), ("identb", (128, 128), bf16),
]
PHASE_WEIGHTS = {"w1", "w2", "wqkv", "wpm", "wkq", "wkkv", "wpk",
                 "bqkv", "b1r", "b2r", "ones", "onesb", "condt", "kbw1",
                 "kbw2", "ident", "gm24t"}


def _swap_ap(ap_rope):
    """AP of shape (..., 8) rope dims -> AP reading pair-swapped elements,
    shaped (..., 4, 2)."""
    dims = [list(d) for d in ap_rope.ap]
    assert dims[-1][1] == 8
    step = dims[-1][0]
    new = dims[:-1] + [[2 * step, 4], [-step, 2]]
    return bass_rust.AP(tensor=ap_rope.tensor, offset=ap_rope.offset + step, ap=new)


def build_nc(n_tiles=N_TILES, phases=(0, 1, 2, 3), work_tiles=None):
    nc = bacc.Bacc("TRN2", target_bir_lowering=False, debug=False, num_devices=8)
    xt = nc.dram_tensor("xt", [n_tiles * ROWS, C], f32, kind="ExternalInput").ap()
    xtt = nc.dram_tensor("xtt", [n_tiles, 128, 12, 128], f32, kind="ExternalInput").ap()
    cst = {}
    for name, shp, dt in CONST_SPECS:
        cst[name] = nc.dram_tensor(name, list(shp), dt, kind="ExternalInput").ap()
    out_d = nc.dram_tensor("out", [n_tiles * ROWS, C], f32, kind="ExternalOutput").ap()
    xu_d = nc.dram_tensor("xu", [n_tiles * ROWS, C], bf16, kind="Internal").ap()
    with tile.TileContext(nc) as tc:
        _prog(nc, tc, xt, xtt, cst, out_d, xu_d,
              n_tiles if work_tiles is None else work_tiles, phases)
    nc.compile()
    return nc


def _prog(nc, tc, xt, xtt, cst, out_d, xu_d, n_tiles, phases=(0, 1, 2, 3)):
    from contextlib import ExitStack
    with ExitStack() as ctx:
        pers = ctx.enter_context(tc.tile_pool(name="pers", bufs=1))
        pp_mm = ctx.enter_context(tc.tile_pool(name="ppmm", bufs=3, space="PSUM"))
        pp_tp = ctx.enter_context(tc.tile_pool(name="pptp", bufs=2, space="PSUM"))
        pp_sc = ctx.enter_context(tc.tile_pool(name="ppsc", bufs=3, space="PSUM"))

        # ---- persistent consts ----
        k = {}
        for name, shp, dt in CONST_SPECS:
            if name in PHASE_WEIGHTS:
                continue
            t_ = pers.tile(list(shp), dt, tag=name)
            nc.sync.dma_start(out=t_, in_=cst[name])
            k[name] = t_
        gm24t_r = pers.tile([24, ROWS], f32r, tag="gm24t_r")
        nc.sync.dma_start(out=gm24t_r, in_=cst["gm24t"].bitcast(f32r))
        ones_r = pers.tile([1, 128], f32r, tag="ones_r")
        nc.sync.dma_start(out=ones_r, in_=cst["ones"].bitcast(f32r))
        condt_r = pers.tile([6, NF], f32, tag="condt_r")
        nc.sync.dma_start(out=condt_r, in_=cst["condt"])
        kbw1_r = pers.tile([6, HID], f32, tag="kbw1_r")
        nc.sync.dma_start(out=kbw1_r, in_=cst["kbw1"])
        kbw2_r = pers.tile([HID, HID], f32, tag="kbw2_r")
        nc.sync.dma_start(out=kbw2_r, in_=cst["kbw2"])

        eps6 = pers.tile([128, 1], f32, tag="eps6")
        nc.vector.memset(eps6, 1e-6)
        eps5 = pers.tile([128, 1], f32, tag="eps5")
        nc.vector.memset(eps5, 1e-5)

        h_all = pers.tile([ROWS, n_tiles, CM], bf16, tag="h_all")
        k2bd = pers.tile([128, 8, 2 * T], bf16, tag="k2bd")
        v2bd = pers.tile([128, 2, 512], bf16, tag="v2bd")

        def trans_copy(src, dst, ident_t, psum_dt, n_chunks):
            for kk in range(n_chunks):
                ps = pp_tp.tile([128, 128], psum_dt, tag="tp")
                nc.tensor.transpose(ps, src[:, kk * 128:(kk + 1) * 128], ident_t)
                nc.vector.tensor_copy(out=dst[:, kk, :], in_=ps)

        def rms_stage1(scr, q_ps, cg, tg):
            """sq + CG-mult; the only psum readers. Returns (sq, qn, P)."""
            P = q_ps.shape[0]
            HH = 8
            sq = scr.tile([P, 512], bf16, tag="sq" + tg)
            nc.scalar.square(out=sq, in_=q_ps)
            qn = scr.tile([P, 512], bf16, tag="qn" + tg)
            nc.vector.scalar_tensor_tensor(
                out=qn.rearrange("p (h d) -> p h d", h=HH),
                in0=q_ps.rearrange("p (h d) -> p h d", h=HH), scalar=1.0,
                in1=cg.rearrange("p (o d) -> p o d", o=1).broadcast_to([P, HH, HD]),
                op0=Alu.mult, op1=Alu.mult)
            return sq, qn, P

        def rms_stage2(scr, st, tanx, out_half, tg):
            """reduce -> rsqrt -> tan-rope on qn -> apply rstd."""
            sq, qn, P = st
            HH = 8
            qn3 = qn.rearrange("p (h d) -> p h d", h=HH)
            ss = scr.tile([P, HH], f32, tag="ss" + tg)
            nc.vector.tensor_reduce(out=ss, in_=sq.rearrange("p (h d) -> p h d", h=HH),
                                    axis=AxX, op=Alu.add)
            rt = scr.tile([P, HH], f32, tag="rt" + tg)
            nc.scalar.activation(out=rt, in_=ss, func=Act.Ln,
                                 bias=eps6[:P], scale=1.0 / HD)
            rq = scr.tile([P, HH], f32, tag="rq" + tg)
            nc.scalar.activation(out=rq, in_=rt, func=Act.Exp, scale=-0.5)
            # rope correction (first 8 dims/head): tmp = swap(qn) * tanx
            # (even/odd strided 3D ops; >3D DVE ops are rejected)
            tmp = scr.tile([P, HH, 8], bf16, tag="tmp" + tg)

            def _ev(ap, off):
                dims = [list(d) for d in ap.ap]
                step = dims[-1][0]
                nd = dims[:-1] + [[2 * step, 4]]
                return bass_rust.AP(tensor=ap.tensor, offset=ap.offset + off * step,
                                    ap=nd)

            q3r = qn3[:, :, 0:8]
            tmp3 = tmp[:, :, 0:8]
            tx = tanx[:P].rearrange("p (h d) -> p h d", h=HH)
            for off in (0, 1):
                nc.vector.scalar_tensor_tensor(
                    out=_ev(tmp3, off), in0=_ev(q3r, 1 - off), scalar=1.0,
                    in1=_ev(tx, off), op0=Alu.mult, op1=Alu.mult)
            nc.vector.tensor_tensor(out=qn3[:, :, 0:8], in0=qn3[:, :, 0:8], in1=tmp,
                                    op=Alu.add)
            nc.vector.tensor_tensor(
                out=out_half.rearrange("p (h d) -> p h d", h=HH),
                in0=qn3,
                in1=rq.rearrange("p (h o) -> p h o", o=1).broadcast_to([P, HH, HD]),
                op=Alu.mult)

        def rms_rope_half(scr, q_ps, cg, tanx, out_half, tg):
            st = rms_stage1(scr, q_ps, cg, tg)
            rms_stage2(scr, st, tanx, out_half, tg)

        def rms_rope(scr, q_ps2, cg, tanx, out_tile, tg=""):
            if not isinstance(q_ps2, (list, tuple)):
                q_ps2 = [q_ps2[:, 0:512], q_ps2[:, 512:1024]]
            for i in range(2):
                rms_rope_half(scr, q_ps2[i], cg, tanx,
                              out_tile[:, i * 512:(i + 1) * 512], tg + str(i))

        # ================= P0: keyboard k2/v2 =================
        if 0 not in phases:
            return
        with tc.tile_pool(name="p0w", bufs=1) as p0w:
            wkkv_s = p0w.tile([128, 12, 2 * CK], f32r, tag="wkkv")
            for kk in range(12):
                nc.sync.dma_start(out=wkkv_s[:, kk, :],
                                  in_=cst["wkkv"][kk * 128:(kk + 1) * 128, :].bitcast(f32r))
            ps0 = pp_mm.tile([HID, NF], f32, tag="mm")
            nc.tensor.matmul(ps0, kbw1_r, condt_r, start=True, stop=True)
            kb1 = p0w.tile([HID, NF], f32, tag="kb1")
            nc.scalar.activation(out=kb1, in_=ps0, func=Act.Silu,
                                 bias=k["kb1c"], scale=1.0)
            ps1 = pp_mm.tile([HID, NF], f32, tag="mm")
            nc.tensor.matmul(ps1, kbw2_r, kb1, start=True, stop=True)
            kb2 = p0w.tile([HID, NF], f32, tag="kb2")
            nc.vector.tensor_scalar(out=kb2, in0=ps1, scalar1=k["kb2c"], scalar2=None,
                                    op0=Alu.add)
            gkt = p0w.tile([HID, 12, T], f32r, tag="gkt")
            for w in range(12):
                t0 = (12 - w + 3) // 4  # ceil((12-w)/4)
                if t0 > 0:
                    nc.vector.tensor_copy(out=gkt[:, w, 0:t0],
                                          in_=kb2[:, 0:1].broadcast_to([HID, t0]))
                start = 4 * t0 + w - 12
                src = bass_rust.AP(tensor=kb2.tensor, offset=kb2.offset + start,
                                   ap=[list(kb2.ap[0]), [4, T - t0]])
                nc.vector.tensor_copy(out=gkt[:, w, t0:T], in_=src)
            kv_s = p0w.tile([T, 2 * CK], f32, tag="kv_s")
            for n in range(4):
                ps = pp_mm.tile([T, 512], f32, tag="mm")
                for w in range(12):
                    nc.tensor.matmul(ps, gkt[:, w, :],
                                     wkkv_s[:, w, n * 512:(n + 1) * 512],
                                     start=(w == 0), stop=(w == 11))
                nc.vector.tensor_copy(out=kv_s[:, n * 512:(n + 1) * 512], in_=ps)
            k2n = p0w.tile([T, CK], bf16, tag="k2n")
            rms_rope(p0w, kv_s[:, 0:CK], k["cg16"], k["tanx16"], k2n, tg="p0")
            nc.vector.memset(k2bd, 0.0)
            for kk in range(8):
                ps = pp_tp.tile([128, T], bf16, tag="tp")
                nc.tensor.transpose(ps, k2n[:, kk * 128:(kk + 1) * 128],
                                    k["identb"][:T, :T])
                nc.vector.tensor_copy(out=k2bd[0:HD, kk, 0:T], in_=ps[0:HD, :])
                nc.vector.tensor_copy(out=k2bd[HD:128, kk, T:2 * T], in_=ps[HD:128, :])
            nc.vector.memset(v2bd, 0.0)
            v2b_t = p0w.tile([T, CK], bf16, tag="v2b_t")
            nc.vector.tensor_copy(out=v2b_t, in_=kv_s[:, CK:2 * CK])
            for h in range(H):
                g, hh = h // 8, h % 8
                nc.sync.dma_start(
                    out=v2bd[hh * T:(hh + 1) * T, g, hh * HD:(hh + 1) * HD],
                    in_=v2b_t[:, h * HD:(h + 1) * HD])

        # ================= P1: mouse MLP + LN -> h_all =================
        if 1 not in phases:
            return
        with tc.tile_pool(name="p1w", bufs=1) as p1w, \
             tc.tile_pool(name="p1a", bufs=2) as p1a, \
             tc.tile_pool(name="p1b", bufs=2) as p1b:
            w1_s = p1w.tile([128, 12, CM], f32r, tag="w1")
            for kk in range(12):
                nc.sync.dma_start(out=w1_s[:, kk, :],
                                  in_=cst["w1"][kk * 128:(kk + 1) * 128, :].bitcast(f32r))
            w1b_s = p1w.tile([24, CM], f32r, tag="w1b")
            nc.sync.dma_start(out=w1b_s, in_=cst["w1"][1536:1560, :].bitcast(f32r))
            w2_s = p1w.tile([128, 8, CM], bf16, tag="w2")
            for kk in range(8):
                nc.sync.dma_start(out=w2_s[:, kk, :],
                                  in_=cst["w2"][kk * 128:(kk + 1) * 128, :])
            b1r_s = p1w.tile([1, CM], f32r, tag="b1r")
            nc.sync.dma_start(out=b1r_s, in_=cst["b1r"].bitcast(f32r))
            b2r_s = p1w.tile([1, CM], bf16, tag="b2r")
            nc.sync.dma_start(out=b2r_s, in_=cst["b2r"])
            onesb1_s = p1w.tile([1, 128], bf16, tag="onesb1")
            nc.sync.dma_start(out=onesb1_s, in_=cst["onesb"])

            for j in range(n_tiles):
                xT = p1a.tile([128, 12, 128], f32r, tag="xT")
                nc.sync.dma_start(out=xT, in_=xtt[j].bitcast(f32r))
                h1 = p1a.tile([ROWS, CM], bf16, tag="h1")
                for n in range(2):
                    sl = slice(n * 512, (n + 1) * 512)
                    ps1t = pp_mm.tile([ROWS, 512], f32, tag="mm")
                    for kk in range(12):
                        nc.tensor.matmul(ps1t, xT[:, kk, :], w1_s[:, kk, sl],
                                         start=(kk == 0), stop=False)
                    nc.tensor.matmul(ps1t, gm24t_r, w1b_s[:, sl],
                                     start=False, stop=False)
                    nc.tensor.matmul(ps1t, ones_r, b1r_s[:, sl],
                                     start=False, stop=True)
                    nc.scalar.activation(out=h1[:, sl], in_=ps1t,
                                         func=Act.Gelu_apprx_tanh)
                h1T = p1b.tile([128, 8, 128], bf16, tag="h1T")
                trans_copy(h1, h1T, k["identb"], bf16, 8)
                stats = p1b.tile([ROWS, 2, 6], f32, tag="stats")
                ps2h = []
                for n in range(2):
                    sl = slice(n * 512, (n + 1) * 512)
                    ps2 = pp_mm.tile([ROWS, 512], f32, tag="mm")
                    for kk in range(8):
                        nc.tensor.matmul(ps2, h1T[:, kk, :], w2_s[:, kk, sl],
                                         start=(kk == 0), stop=False)
                    nc.tensor.matmul(ps2, onesb1_s, b2r_s[:, sl],
                                     start=False, stop=True)
                    nc.vector.bn_stats(out=stats[:, n, :], in_=ps2)
                    ps2h.append(ps2)
                mv = p1b.tile([ROWS, 2], f32, tag="mv")
                nc.vector.bn_aggr(out=mv, in_=stats)
                sd = p1b.tile([ROWS, 1], f32, tag="sd")
                nc.scalar.activation(out=sd, in_=mv[:, 1:2], func=Act.Ln,
                                     bias=eps5, scale=1.0)
                rstd = p1b.tile([ROWS, 1], f32, tag="rstd")
                nc.scalar.activation(out=rstd, in_=sd, func=Act.Exp, scale=-0.5)
                for n in range(2):
                    nc.vector.tensor_scalar(
                        out=h_all[:, j, n * 512:(n + 1) * 512], in0=ps2h[n],
                        scalar1=mv[:, 0:1], scalar2=rstd,
                        op0=Alu.subtract, op1=Alu.mult)

        # ================= P2: qkv + mouse attention + proj -> xu =================
        if 2 not in phases:
            return
        with tc.tile_pool(name="p2w", bufs=1) as p2w, \
             tc.tile_pool(name="p2s", bufs=2) as p2s, \
             tc.tile_pool(name="p2c", bufs=2) as p2c, \
             tc.tile_pool(name="p2a", bufs=2) as p2a, \
             tc.tile_pool(name="p2b", bufs=3) as p2b:
            wqkv_s = p2w.tile([128, 8, 3 * CM], bf16, tag="wqkv")
            for kk in range(8):
                nc.sync.dma_start(out=wqkv_s[:, kk, :],
                                  in_=cst["wqkv"][kk * 128:(kk + 1) * 128, :])
            bqkv_s = p2w.tile([1, 3 * CM], bf16, tag="bqkv")
            nc.sync.dma_start(out=bqkv_s, in_=cst["bqkv"])
            onesb_s = p2w.tile([1, 128], bf16, tag="onesb_s")
            nc.sync.dma_start(out=onesb_s, in_=cst["onesb"])
            wpm_s = p2w.tile([128, 8, C], bf16, tag="wpm")
            for kk in range(8):
                nc.sync.dma_start(out=wpm_s[:, kk, :],
                                  in_=cst["wpm"][kk * 128:(kk + 1) * 128, :])

            for j in range(n_tiles):
                hT = p2c.tile([128, 8, 128], bf16, tag="hT")
                trans_copy(h_all[:, j, :], hT, k["identb"], bf16, 8)
                x_s = p2a.tile([ROWS, C], f32, tag="x_s2")
                nc.sync.dma_start(out=x_s, in_=xt[j * ROWS:(j + 1) * ROWS, :])

                def qkv_half(part, n):
                    sl_o = slice(part * CM + n * 512, part * CM + (n + 1) * 512)
                    ps = pp_mm.tile([ROWS, 512], f32, tag="mm")
                    for kk in range(8):
                        nc.tensor.matmul(ps, hT[:, kk, :], wqkv_s[:, kk, sl_o],
                                         start=(kk == 0), stop=False)
                    nc.tensor.matmul(ps, onesb_s, bqkv_s[:, sl_o],
                                     start=False, stop=True)
                    return ps

                qn = p2c.tile([ROWS, CM], bf16, tag="qnb")
                kn = p2c.tile([ROWS, CM], bf16, tag="knb")
                v_s = p2c.tile([ROWS, CM], bf16, tag="v_s")
                halves = [(0, 0, k["cgq"], qn), (0, 1, k["cgq"], qn),
                          (1, 0, k["cgk"], kn), (1, 1, k["cgk"], kn)]
                sts = []
                for i, (part, n, cg, _o) in enumerate(halves):
                    ps = qkv_half(part, n)
                    sts.append(rms_stage1(p2s, ps, cg, "h%d" % i))
                for i, (part, n, _c, outt) in enumerate(halves):
                    rms_stage2(p2s, sts[i], k["tanx"],
                               outt[:, n * 512:(n + 1) * 512], "h%d" % i)
                for n in range(2):
                    ps = qkv_half(2, n)
                    nc.vector.tensor_copy(out=v_s[:, n * 512:(n + 1) * 512], in_=ps)

                qT = p2b.tile([128, 8, 128], bf16, tag="qT")
                trans_copy(qn, qT, k["identb"], bf16, 8)
                kT = p2b.tile([128, 8, 128], bf16, tag="kT")
                trans_copy(kn, kT, k["identb"], bf16, 8)

                aoT = p2c.tile([128, 8, 128], bf16, tag="aoT")
                # software-pipelined head loop (depth 2). Probs are pre-scaled
                # by 1/rowsum before the transpose; PV computes the attention
                # output directly transposed (lhsT = v slice), pairs of heads
                # stacked on partitions so one copy drains two heads.
                sc_l, es_l, sum_l, pv_l = {}, {}, {}, {}

                def stage_scores(h):
                    n_sl, p_off = h // 2, (h % 2) * HD
                    sc = pp_sc.tile([ROWS, ROWS], f32, tag="sc")
                    nc.tensor.matmul(sc, qT[p_off:p_off + HD, n_sl, :],
                                     kT[p_off:p_off + HD, n_sl, :],
                                     start=True, stop=True)
                    nc.vector.scalar_tensor_tensor(out=sc, in0=sc, scalar=SCALE,
                                                   in1=k["madd"],
                                                   op0=Alu.mult, op1=Alu.add)
                    sc_l[h] = sc

                def stage_exp(h):
                    e_s = p2b.tile([ROWS, ROWS], bf16, tag="e_s")
                    esum = p2b.tile([ROWS, 1], f32, tag="esum")
                    nc.scalar.activation(out=e_s, in_=sc_l.pop(h), func=Act.Exp,
                                         scale=1.0, accum_out=esum)
                    es_l[h], sum_l[h] = e_s, esum

                def stage_out(h):
                    erec = p2b.tile([ROWS, 1], f32, tag="erec")
                    nc.vector.reciprocal(out=erec, in_=sum_l.pop(h))
                    e_c = p2b.tile([ROWS, ROWS], bf16, tag="e_c")
                    nc.vector.tensor_scalar(out=e_c, in0=es_l.pop(h),
                                            scalar1=erec, scalar2=None, op0=Alu.mult)
                    pt_ps = pp_tp.tile([128, 128], bf16, tag="tp")
                    nc.tensor.transpose(pt_ps, e_c, k["identb"])
                    pt_s = p2b.tile([128, 128], bf16, tag="pt_s")
                    nc.vector.tensor_copy(out=pt_s, in_=pt_ps)
                    if h % 2 == 0:
                        pv_l[h // 2] = pp_sc.tile([128, 128], f32, tag="sc",
                                                   name="pv%d" % (h // 2))
                    pv = pv_l[h // 2]
                    p_off = (h % 2) * HD
                    nc.tensor.matmul(pv[p_off:p_off + HD, :],
                                     v_s[:, h * HD:(h + 1) * HD], pt_s,
                                     start=True, stop=True)
                    if h % 2 == 1:
                        nc.vector.tensor_copy(out=aoT[:, h // 2, :],
                                              in_=pv_l.pop(h // 2))

                for h in range(H + 2):
                    if h < H:
                        stage_scores(h)
                    if 1 <= h <= H:
                        stage_exp(h - 1)
                    if h >= 2:
                        stage_out(h - 2)
                xu_s = p2a.tile([ROWS, C], bf16, tag="xu_s")
                for n in range(3):
                    sl = slice(n * 512, (n + 1) * 512)
                    psp = pp_mm.tile([ROWS, 512], f32, tag="mm")
                    for kk in range(8):
                        nc.tensor.matmul(psp, aoT[:, kk, :], wpm_s[:, kk, sl],
                                         start=(kk == 0), stop=(kk == 7))
                    nc.vector.tensor_tensor(out=xu_s[:, sl], in0=psp,
                                            in1=x_s[:, sl], op=Alu.add)
                nc.sync.dma_start(out=xu_d[j * ROWS:(j + 1) * ROWS, :], in_=xu_s)

        # ================= P3: keyboard attention + proj -> out =================
        if 3 not in phases:
            return
        with tc.tile_pool(name="p3w", bufs=1) as p3w, \
             tc.tile_pool(name="p3s", bufs=2) as p3s, \
             tc.tile_pool(name="p3c", bufs=2) as p3c, \
             tc.tile_pool(name="p3a", bufs=2) as p3a, \
             tc.tile_pool(name="p3b", bufs=3) as p3b:
            wkq_s = p3w.tile([128, 12, CK], bf16, tag="wkq")
            for kk in range(12):
                nc.sync.dma_start(out=wkq_s[:, kk, :],
                                  in_=cst["wkq"][kk * 128:(kk + 1) * 128, :])
            wpk_s = p3w.tile([128, 8, C], bf16, tag="wpk")
            for kk in range(8):
                nc.sync.dma_start(out=wpk_s[:, kk, :],
                                  in_=cst["wpk"][kk * 128:(kk + 1) * 128, :])

            def p3_s1(j):
                xu_s = p3a.tile([ROWS, C], bf16, tag="xu_s3")
                nc.sync.dma_start(out=xu_s, in_=xu_d[j * ROWS:(j + 1) * ROWS, :])
                xuT = p3c.tile([128, 12, 128], bf16, tag="xuT")
                trans_copy(xu_s, xuT, k["identb"], bf16, 12)
                q2n = p3c.tile([ROWS, CK], bf16, tag="q2n")
                sts = []
                for n in range(2):
                    sl = slice(n * 512, (n + 1) * 512)
                    q2_ps = pp_mm.tile([ROWS, 512], f32, tag="mm")
                    for kk in range(12):
                        nc.tensor.matmul(q2_ps, xuT[:, kk, :], wkq_s[:, kk, sl],
                                         start=(kk == 0), stop=(kk == 11))
                    sts.append(rms_stage1(p3s, q2_ps, k["cgq2"], "q2%d" % n))
                for n in range(2):
                    rms_stage2(p3s, sts[n], k["tanx"],
                               q2n[:, n * 512:(n + 1) * 512], "q2%d" % n)
                q2T = p3c.tile([128, 8, 128], bf16, tag="q2T")
                trans_copy(q2n, q2T, k["identb"], bf16, 8)
                return xu_s, q2T

            def p3_s2(j, q2T):
                sm2 = p3b.tile([ROWS, H, T], f32, tag="sm2")
                for pr in range(H // 2):
                    sc2 = pp_sc.tile([ROWS, 2, T], f32, tag="sc")
                    nc.tensor.matmul(sc2.rearrange("p a t -> p (a t)"),
                                     q2T[:, pr, :], k2bd[:, pr, :],
                                     start=True, stop=True)
                    nc.vector.scalar_tensor_tensor(
                        out=sm2[:, 2 * pr:2 * pr + 2, :], in0=sc2, scalar=SCALE,
                        in1=k["madd2"].rearrange("p (o t) -> p o t", o=1)
                            .broadcast_to([ROWS, 2, T]),
                        op0=Alu.mult, op1=Alu.add)
                e2e = p3b.tile([ROWS, H, T], f32, tag="e2e")
                nc.scalar.activation(out=e2e, in_=sm2, func=Act.Exp)
                s2 = p3b.tile([ROWS, H], f32, tag="s2")
                nc.vector.tensor_reduce(out=s2, in_=e2e, axis=AxX, op=Alu.add)
                r2 = p3b.tile([ROWS, H], f32, tag="r2")
                nc.vector.reciprocal(out=r2, in_=s2)
                p2_t = p3b.tile([ROWS, H, T], bf16, tag="p2_t")
                nc.vector.tensor_tensor(
                    out=p2_t, in0=e2e,
                    in1=r2.rearrange("p (h o) -> p h o", o=1).broadcast_to([ROWS, H, T]),
                    op=Alu.mult)
                return p2_t

            def p3_s3(j, xu_s, p2_t):
                # out2 computed directly transposed: o2T chunk (2 heads' d, tok)
                #   = v2bd_slice.T @ pstk
                o2T = p3b.tile([128, 8, 128], bf16, tag="o2T")
                for g in range(2):
                    pstk = p3b.tile([128, 128], bf16, tag="pstk")
                    for hh in range(4):
                        ptp = pp_tp.tile([2 * T, 128], bf16, tag="tp")
                        nc.tensor.transpose(
                            ptp,
                            p2_t[:, 2 * g * 4 + 2 * hh:2 * g * 4 + 2 * hh + 2, :]
                                .rearrange("p a t -> p (a t)"),
                            k["identb"])
                        nc.vector.tensor_copy(out=pstk[hh * 32:(hh + 1) * 32, :],
                                              in_=ptp)
                    for c2 in range(4):
                        ops = pp_sc.tile([128, 128], f32, tag="sc")
                        nc.tensor.matmul(ops, v2bd[:, g, c2 * 128:(c2 + 1) * 128],
                                         pstk, start=True, stop=True)
                        nc.vector.tensor_copy(out=o2T[:, g * 4 + c2, :], in_=ops)
                fin = p3a.tile([ROWS, C], f32, tag="fin")
                for n in range(3):
                    sl = slice(n * 512, (n + 1) * 512)
                    psp = pp_mm.tile([ROWS, 512], f32, tag="mm")
                    for kk in range(8):
                        nc.tensor.matmul(psp, o2T[:, kk, :], wpk_s[:, kk, sl],
                                         start=(kk == 0), stop=(kk == 7))
                    nc.vector.tensor_tensor(out=fin[:, sl], in0=psp,
                                            in1=xu_s[:, sl], op=Alu.add)
                nc.sync.dma_start(out=out_d[j * ROWS:(j + 1) * ROWS, :], in_=fin)

            for j in range(n_tiles):
                xu_s, q2T = p3_s1(j)
                p2_t = p3_s2(j, q2T)
                p3_s3(j, xu_s, p2_t)


_NC_CACHE = {}


def _get_nc(n_tiles=N_TILES):
    if n_tiles not in _NC_CACHE:
        _NC_CACHE[n_tiles] = build_nc(n_tiles)
    return _NC_CACHE[n_tiles]


def _permute_x(x):
    x3 = np.asarray(x, np.float32).reshape(T, S, C)
    s_idx = np.array([[s0 + u for u in range(SCH)] for s0 in S_STARTS])
    shards = []
    for c in range(8):
        g = x3[:, c * SPC + s_idx, :]          # (T, 25, 8, C)
        g = np.ascontiguousarray(g.transpose(1, 0, 2, 3).reshape(N_TILES * ROWS, C))
        shards.append(g)
    return shards


def _unpermute_out(outs):
    res = np.empty((T, S, C), np.float32)
    j_of_s = np.minimum(np.arange(SPC) // SCH, N_TILES - 1)
    u_of_s = np.arange(SPC) - np.array(S_STARTS)[j_of_s]
    for c in range(8):
        o = np.asarray(outs[c], np.float32).reshape(N_TILES, T, SCH, C)
        res[:, c * SPC:(c + 1) * SPC, :] = o[j_of_s, :, u_of_s, :].transpose(1, 0, 2)
    return res.reshape(1, T * S, C)


def _in_maps(inputs):
    consts = _build_consts(inputs)
    shards = _permute_x(inputs["x"])
    np_dt = {f32: np.float32, bf16: ml_dtypes.bfloat16}
    in_maps = []
    for c in range(8):
        xtt = np.ascontiguousarray(
            shards[c].reshape(N_TILES, ROWS, 12, 128).transpose(0, 3, 2, 1))
        m = {"xt": shards[c], "xtt": xtt}
        for name, shp, dt in CONST_SPECS:
            m[name] = np.ascontiguousarray(
                np.asarray(consts[name]).astype(np_dt[dt]).reshape(shp))
        in_maps.append(m)
    return in_maps


def run(inputs, trace=False):
    nc = _get_nc()
    res = run_bass_kernel_spmd(nc, _in_maps(inputs), core_ids=list(range(8)),
                               trace=trace)
    return _unpermute_out([r["out"] for r in res.results]), res


def kernel(**inputs):
    out, _ = run(inputs)
    return out


if __name__ == "__main__":
    import time
    t0 = time.time()
    nc = build_nc()
    n_inst = sum(len(f.bbs[0].instructions) if hasattr(f, 'bbs') else 0
                 for f in nc.m.functions)
    print("build+compile time:", time.time() - t0)

